# revision 50
# baseline (speedup 1.0000x reference)
"""AdjustHueSaturation Trainium2 kernel (fp16 pipeline).

Full inputs: imgs (64,3,512,512) f32 in [0,1], xform_params (64,2) f32
(hue delta in [-0.5,0.5], sat scale in [0.2,2]).  Output: (64,3,512,512) f32.

Sharding: pure batch data-parallel across 8 NeuronCores (8 images/core).
The host converts images to fp16 before upload and back to f32 after
download (halves HBM traffic; fp16 keeps worst-case error ~5e-3, well
under the 2e-2 gate — validated in proto.py against the jax reference).

Per-pixel math (per image, host precomputes ds and A'_j):
    A'_j = mod(6*dh + 2j + 3, 6) - 3   in [-3,3),  j=0,1,2 (r,g,b branch)
    maxc, minc, cr = max, min, chroma            (DVE tensor_tensor, fp16 2x)
    icr  = exp(-ln(cr + 3e-5))                   (ScalarE Ln/Exp, one table set;
                                                  ln kept fp32 for icr accuracy)
    crA_j = cr*A'_j, crds = cr*ds                (ScalarE Copy with [P,1] scale)
    c    = min(crds, maxc)
    d1=g-b, d2=b-r, d3=r-g     (host ships planes in (g,b,r) order)
    n    = crA_0 + d1, overwritten by crA_1 + d2 where g>r (mask = ScalarE
           uint16(relu(-d3*2^14))), then by crA_2 + d3 where b>max(r,g)
    m6   = wrap(n*icr) = w + (w>=0 ? -3 : 3) in [-3,3)   # custom DVE op
    x_ch = clamp((|m6 + k_ch| - 1)*c, 0, c), k = 0/+1/-1 # custom DVE op;
           red variant emits c - x so all channels share one final form
    out3 = maxc (page-broadcast) - x3            (single [P,3,FD] subtract)
Matches the reference up to fp16 rounding (branch ties are continuous, so
mask misfires within a tie neighborhood are harmless).
"""

import numpy as np

B, C, H, W = 64, 3, 512, 512
N_CORES = 8
IPC = B // N_CORES          # images per core
P = 128                     # SBUF partitions
FDTOT = (H * W) // P        # 2048 elems per partition per plane
FD = 2048                   # free-dim chunk per tile (full plane per image)
NCH = FDTOT // FD

_cache = {}


def _register_op(name, spec):
    import concourse.dve_ops as dvo
    from concourse.dve_spec import lower, spec_leaves, Src1
    from concourse.dve_ops import DveOp, DveOpSpec, has_src1

    rd1 = has_src1(spec)
    shas = {}
    for ver in ("v3", "v4"):
        shas[ver] = DveOpSpec(
            name=name, opcode=0, uops=lower(spec, ver=ver), rd1_en=rd1
        ).sha(ver)
    op = DveOp(name, spec, subdim=False, uops_sha=shas)
    dvo.OPS.append(op)
    dvo.CUSTOM_DVE_SPECS[op.name] = op.spec
    dvo._SUB_OPCODE_FOR_NAME[op.name] = dvo._CUSTOM_DVE_ROW_BASE + len(dvo.OPS) - 1
    assert dvo._SUB_OPCODE_FOR_NAME[op.name] < 0x20
    return op


def _custom_ops():
    """Register the fused DVE ops (once per process)."""
    if "ops" in _cache:
        return _cache["ops"]
    from concourse.dve_spec import Spec, Src0, Src1, C0, C1, Zero, One, maxx, minn, select

    def _tri(x, k):
        tk = x + k
        a = maxx(tk, Zero - tk)
        return minn(maxx((a - One) * Src1, Zero), Src1)

    # x = clamp((|m6 + s0| - 1) * c, 0, c)
    tri = _register_op("HSV_TRI", Spec(
        body=_tri(Src0, C0),
        reference=lambda in0, in1, s0: np.minimum(
            np.maximum((np.abs(in0 + s0) - 1.0) * in1, 0.0), in1),
    ))
    # c - clamp(...) (red channel: o_r = maxc - (c - x_r))
    tri_r = _register_op("HSV_TRI_R", Spec(
        body=Src1 - _tri(Src0, C0),
        reference=lambda in0, in1, s0: in1 - np.minimum(
            np.maximum((np.abs(in0 + s0) - 1.0) * in1, 0.0), in1),
    ))
    # m6 = wrap(n*icr): w = in0*in1 in [-4,4]; m6 = w + (w>=0 ? s0 : s1)
    wrapmul = _register_op("HSV_WRAPMUL", Spec(
        body=(lambda w: w + select(w >= Zero, C0, C1))(Src0 * Src1),
        reference=lambda in0, in1, s0, s1: (lambda w: w + np.where(w >= 0, s0, s1))(in0 * in1),
    ))
    # negated variants: DMA-accumulate adds these onto a maxc-prefilled output
    tri_neg = _register_op("HSV_TRI_NEG", Spec(
        body=Zero - _tri(Src0, C0),
        reference=lambda in0, in1, s0: -np.minimum(
            np.maximum((np.abs(in0 + s0) - 1.0) * in1, 0.0), in1),
    ))
    tri_r_neg = _register_op("HSV_TRI_R_NEG", Spec(
        body=_tri(Src0, C0) - Src1,
        reference=lambda in0, in1, s0: np.minimum(
            np.maximum((np.abs(in0 + s0) - 1.0) * in1, 0.0), in1) - in1,
    ))
    _cache["ops"] = (tri, tri_r, wrapmul, tri_neg, tri_r_neg)
    return _cache["ops"]


def _patch_act_tables():
    """Force Ln+Exp to resolve to the one table set containing both, so the
    table load hoists out of the loop instead of thrashing every iteration."""
    if _cache.get("act_patched"):
        return
    import concourse.bacc as bacc_mod
    orig = bacc_mod.get_activation_tables

    def patched(arch):
        tables = orig(arch)
        keep = "natural_log_exp_and_others"
        out = {}
        for name, fns in tables.items():
            if name != keep:
                fns = {f for f in fns if str(getattr(f, "name", f)).lower() not in ("ln", "exp")}
            out[name] = fns
        return out

    bacc_mod.get_activation_tables = patched
    _cache["act_patched"] = True


def _build_nc():
    from concourse import bass, bacc, mybir
    from concourse.tile import TileContext

    tri, tri_r, wrapmul, tri_neg, tri_r_neg = _custom_ops()
    _patch_act_tables()

    f16 = mybir.dt.float16
    f32 = mybir.dt.float32
    Alu = mybir.AluOpType
    Act = mybir.ActivationFunctionType

    nc = bacc.Bacc()
    # const AP for the Ln bias (activation converts float bias to a const AP)
    t_ = nc.alloc_sbuf_tensor("const-lnbias", [128, 1], f32)
    nc.gpsimd.memset(t_.ap(), 3e-5)
    nc.const_aps.aps[(f32, 3e-5)] = t_.ap()
    nc.all_engine_barrier()

    imgs_d = nc.declare_dram_parameter("imgs", [IPC * 3, P, FDTOT], f16, isOutput=False)
    scal_d = nc.declare_dram_parameter("scal", [P, 4 * IPC], f32, isOutput=False)
    out_d = nc.declare_dram_parameter("out", [IPC * 3, P, FDTOT], f16, isOutput=True)

    with TileContext(nc) as tc:
        with tc.tile_pool(name="const", bufs=1) as cpool, \
             tc.tile_pool(name="single", bufs=1) as spool, \
             tc.tile_pool(name="inp", bufs=3) as ipool, \
             tc.tile_pool(name="work", bufs=2) as pool:
            scal_ld = cpool.tile([P, 4 * IPC], f32, name="scal_ld")
            scal_sb = cpool.tile([P, 4 * IPC], f32, name="scal_sb")
            nc.sync.dma_start(out=scal_ld[:, :], in_=scal_d[:, :])
            nc.vector.tensor_copy(scal_sb[:, :], scal_ld[:, :])
            # warm the Ln/Exp table set during the first input DMA so the
            # ~2.7us table load is off iteration 0's critical path
            warm = cpool.tile([P, 1], f32, name="warm")
            nc.scalar.activation(warm[:, :], scal_sb[:, 0:1], Act.Ln, bias=3e-5)
            nc.scalar.activation(warm[:, :], warm[:, :], Act.Exp, scale=-1.0)

            for img in range(IPC):
                ds_ap = scal_sb[:, 4 * img + 0:4 * img + 1]
                a0_ap = scal_sb[:, 4 * img + 1:4 * img + 2]
                a1_ap = scal_sb[:, 4 * img + 2:4 * img + 3]
                a2_ap = scal_sb[:, 4 * img + 3:4 * img + 4]
                for chk in range(NCH):
                    lo = chk * FD
                    # per-channel plane tiles: compute starts after 2 plane
                    # loads instead of the full 3-plane block (host order g,b,r)
                    in_g = ipool.tile([P, FD], f16, tag="in_g", name="in_g")
                    in_b = ipool.tile([P, FD], f16, tag="in_b", name="in_b")
                    in_r = ipool.tile([P, FD], f16, tag="in_r", name="in_r")
                    nc.sync.dma_start(out=in_g[:, :], in_=imgs_d[3 * img + 0, :, lo:lo + FD])
                    nc.sync.dma_start(out=in_r[:, :], in_=imgs_d[3 * img + 2, :, lo:lo + FD])
                    nc.sync.dma_start(out=in_b[:, :], in_=imgs_d[3 * img + 1, :, lo:lo + FD])
                    g, b, r = in_g[:, :], in_b[:, :], in_r[:, :]
                    out3 = pool.tile([P, 3, FD], f16, tag="out3", name="out3")

                    def t(tag, dt=f16):
                        return pool.tile([P, FD], dt, tag=tag, name=tag)

                    # tiles are reused across disjoint lifetimes to fit SBUF
                    mxt = t("mxt"); maxc = t("maxc")
                    t1 = t("t1"); t2 = t("t2")
                    cr = t("cr"); icr = t("icr")
                    lncr = spool.tile([P, FD], f32, tag="lncr", name="lncr")
                    crds = t("crds")
                    crA0 = t("crA0"); crA1 = t("crA1"); crA2 = t("crA2")
                    d1 = t("d1"); d2 = t("d2"); d3 = t("d3")
                    m1k = t("m1k", mybir.dt.uint16); m2k = t("m2k", mybir.dt.uint16)
                    mnt, minc = t1, t2
                    c = t1          # after mnt dies
                    n1 = t2         # after minc dies
                    n2, n3 = crds, crA0     # after their producers die
                    m6 = crA1
                    x3 = spool.tile([P, 3, FD], f16, tag="x3", name="x3")

                    # chroma + channel diffs (DVE); d's early so m1k/masks unblock
                    nc.vector.tensor_tensor(mxt[:, :], r, g, Alu.max)
                    nc.vector.tensor_tensor(maxc[:, :], mxt[:, :], b, Alu.max)
                    nc.vector.tensor_tensor(d1[:, :], g, b, Alu.subtract)
                    nc.vector.tensor_tensor(d2[:, :], b, r, Alu.subtract)
                    nc.vector.tensor_tensor(d3[:, :], r, g, Alu.subtract)
                    nc.vector.tensor_tensor(mnt[:, :], r, g, Alu.min)
                    nc.vector.tensor_tensor(minc[:, :], mnt[:, :], b, Alu.min)
                    nc.vector.tensor_tensor(cr[:, :], maxc[:, :], minc[:, :], Alu.subtract)

                    # ScalarE: mask + per-image scalar multiplies (they gate
                    # the DVE n-builds), then 1/cr for the later wrapmul
                    nc.scalar.activation(m1k[:, :], d3[:, :], Act.Relu, scale=-16384.0)
                    nc.scalar.activation(crA0[:, :], cr[:, :], Act.Copy, scale=a0_ap)
                    nc.scalar.activation(crA1[:, :], cr[:, :], Act.Copy, scale=a1_ap)
                    nc.scalar.activation(crA2[:, :], cr[:, :], Act.Copy, scale=a2_ap)
                    nc.scalar.activation(lncr[:, :], cr[:, :], Act.Ln, bias=3e-5)
                    nc.scalar.activation(icr[:, :], lncr[:, :], Act.Exp, scale=-1.0)
                    nc.scalar.activation(crds[:, :], cr[:, :], Act.Copy, scale=ds_ap)

                    nc.vector.tensor_tensor(m2k[:, :], b, mxt[:, :], Alu.is_gt)
                    nc.vector.tensor_tensor(c[:, :], crds[:, :], maxc[:, :], Alu.min)
                    nc.vector.tensor_tensor(n1[:, :], crA0[:, :], d1[:, :], Alu.add)
                    nc.vector.tensor_tensor(n2[:, :], crA1[:, :], d2[:, :], Alu.add)
                    nc.vector.tensor_tensor(n3[:, :], crA2[:, :], d3[:, :], Alu.add)
                    nc.vector.copy_predicated(n1[:, :], m1k[:, :], n2[:, :])
                    nc.vector.copy_predicated(n1[:, :], m2k[:, :], n3[:, :])

                    # m6 = wrap(n/cr) in [-3,3): fused mult + fold
                    nc.vector._custom_dve(wrapmul, out=m6[:, :], in0=n1[:, :], in1=icr[:, :], s0=-3.0, s1=3.0)

                    # x = clamp((|m6+k|-1)*c, 0, c), fused; red emits c-x
                    nc.vector._custom_dve(tri_r, out=x3[:, 0, :], in0=m6[:, :], in1=c[:, :], s0=0.0)
                    nc.vector._custom_dve(tri, out=x3[:, 1, :], in0=m6[:, :], in1=c[:, :], s0=1.0)
                    nc.vector._custom_dve(tri, out=x3[:, 2, :], in0=m6[:, :], in1=c[:, :], s0=-1.0)

                    # out3 = maxc (page-broadcast) - x3; the last iteration is
                    # split into two tiles so the final DMA overlaps the final TT
                    is_last = (img == IPC - 1 and chk == NCH - 1)
                    if not is_last:
                        maxc_b3 = maxc[:, :][:, None, :].to_broadcast([P, 3, FD])
                        nc.vector.tensor_tensor(out3[:, :, :], maxc_b3, x3[:, :, :], Alu.subtract)
                        nc.sync.dma_start(
                            out=out_d[3 * img:3 * img + 3, :, lo:lo + FD].rearrange("c p f -> p c f"),
                            in_=out3[:, :, :])
                    else:
                        H = FD // 2
                        oh2 = spool.tile([P, 3, H], f16, tag="oh2", name="oh2")
                        for h0, h1, dst in ((0, H, out3), (H, FD, oh2)):
                            maxc_bh = maxc[:, h0:h1][:, None, :].to_broadcast([P, 3, H])
                            nc.vector.tensor_tensor(
                                dst[:, :, 0:H] if dst is oh2 else dst[:, :, h0:h1],
                                maxc_bh, x3[:, :, h0:h1], Alu.subtract)
                            nc.sync.dma_start(
                                out=out_d[3 * img:3 * img + 3, :, lo + h0:lo + h1].rearrange("c p f -> p c f"),
                                in_=(dst[:, :, 0:H] if dst is oh2 else dst[:, :, h0:h1]))
    nc.finalize()
    return nc


def _make_in_maps(imgs: np.ndarray, xf: np.ndarray):
    imgs16 = imgs.astype(np.float16)
    dh = xf[:, 0].astype(np.float64)
    sat = xf[:, 1].astype(np.float32)
    A = [(np.mod(6.0 * dh + 2 * j + 3, 6.0) - 3.0).astype(np.float32) for j in range(3)]
    in_maps = []
    for core in range(N_CORES):
        sl = slice(core * IPC, (core + 1) * IPC)
        # device expects channel planes in (g, b, r) order
        shard = imgs16[sl][:, [1, 2, 0]].reshape(IPC * 3, P, FDTOT)
        scal = np.empty((P, 4 * IPC), dtype=np.float32)
        scal[:, 0::4] = sat[sl][None, :]
        scal[:, 1::4] = A[0][sl][None, :]
        scal[:, 2::4] = A[1][sl][None, :]
        scal[:, 3::4] = A[2][sl][None, :]
        in_maps.append({"imgs": shard, "scal": scal})
    return in_maps


def kernel(imgs: np.ndarray, xform_params: np.ndarray) -> np.ndarray:
    from concourse.bass_utils import run_bass_kernel_spmd

    if "nc" not in _cache:
        _cache["nc"] = _build_nc()
    nc = _cache["nc"]

    imgs = np.ascontiguousarray(imgs, dtype=np.float32)
    xf = np.asarray(xform_params, dtype=np.float32)

    in_maps = _make_in_maps(imgs, xf)
    res = run_bass_kernel_spmd(nc, in_maps, core_ids=list(range(N_CORES)))
    out = np.empty((B, C, H, W), dtype=np.float32)
    for core in range(N_CORES):
        out[core * IPC:(core + 1) * IPC] = (
            res.results[core]["out"].astype(np.float32).reshape(IPC, C, H, W))
    return out
